# revision 2
# baseline (speedup 1.0000x reference)
"""MergedEmbeddingBagCat kernel for Trainium2 (8 NeuronCores via axon PJRT).

Strategy: data-parallel over the batch (B=32768 -> 4096 bags/core). All 26
embedding tables are merged into one DRAM tensor (replicated per core);
indices are globalized and packed on the host into a per-(tile,slot)
schedule. Each 128-bag tile runs 214 indirect-gather DMAs (one per
(table, multi-hot j) slot, 128 rows each) that accumulate (CCE add)
directly into a per-tile result block; high multi-hot tables are split
into parallel sub-chains (merged by 4 DVE adds) so no accumulation chain
ever stalls the SWDGE queue. One output DMA per tile writes the final
[128, 27*128] concat rows. Dense features are DMA-copied into block 0.
"""
import numpy as np
import orjson

import concourse.bass as bass
import concourse.mybir as mybir
import concourse.tile as tile

# ---------------------------------------------------------------- problem dims
MULTI_HOT = [3, 2, 1, 2, 6, 1, 1, 1, 1, 7, 3, 8, 1, 6, 9, 5, 1, 1, 1, 12, 100, 27, 10, 3, 1, 1]
POOL = [200000, 39060, 17295, 7424, 20265, 3, 7122, 1543, 63, 200000, 200000, 405282, 10, 2209,
        11938, 155, 4, 976, 14, 200000, 200000, 200000, 590152, 12973, 108, 36]
NUM_TABLES = 26
D = 128
N_CORES = 8
ROW_BASE = np.concatenate([[0], np.cumsum(POOL)]).astype(np.int64)
VTOT = int(ROW_BASE[-1])
NSLOT = sum(MULTI_HOT)          # 214
NOUT = (NUM_TABLES + 1) * D     # 3456

# ---------------------------------------------------------------- walrus patch
# The pinned walrus build allows only ONE sync-wait condition per
# instruction; redistribute extra waits onto single-wait carrier NoOps.
_orig_to_json_bytes = bass.Bass.to_json_bytes


def _split_multiwaits(j):
    for func in j.get("functions", []):
        for blk in func.get("blocks", []):
            out = []
            for inst in blk.get("instructions", []):
                si = inst.get("sync_info")
                waits = (si or {}).get("on_wait") or []
                if len(waits) > 1:
                    for k, w in enumerate(waits[:-1]):
                        out.append({
                            "debug": inst.get("debug", 0),
                            "engine": inst["engine"],
                            "ins": [], "outs": [],
                            "name": f"{inst['name']}-w{k}",
                            "opcode": "NoOp",
                            "sync_info": {"on_update": [], "on_wait": [w]},
                        })
                    si["on_wait"] = [waits[-1]]
                out.append(inst)
            blk["instructions"] = out
    return j


def _patched_to_json_bytes(self):
    return orjson.dumps(_split_multiwaits(orjson.loads(_orig_to_json_bytes(self))))


bass.Bass.to_json_bytes = _patched_to_json_bytes

# ---------------------------------------------------------------- schedule
# Accumulation chains: one per table, except t20 (mh=100) -> 4 sub-chains
# and t21 (mh=27) -> 2 sub-chains, so consecutive instructions in the
# SWDGE queue never target the same destination block back-to-back.
# Block layout in the result tile: 0=dense, 1+t = table t main block,
# 27..29 = t20 extras, 30 = t21 extra.
N_BLK = 31


def make_slots():
    chains = []
    for t, mh in enumerate(MULTI_HOT):
        if t == 20:
            chains += [(t, 0, 25, 1 + t), (t, 25, 25, 27), (t, 50, 25, 28), (t, 75, 25, 29)]
        elif t == 21:
            chains += [(t, 0, 14, 1 + t), (t, 14, 13, 30)]
        else:
            chains.append((t, 0, mh, 1 + t))
    slots = []
    maxlen = max(c[2] for c in chains)
    for step in range(maxlen):
        for (t, j0, ln, blk) in chains:
            if step < ln:
                slots.append((t, j0 + step, blk, step == 0))
    assert len(slots) == NSLOT
    return slots


SLOTS = make_slots()


def build_program(n_tiles):
    nc = bass.Bass()
    w = nc.dram_tensor("w", [VTOT, D], mybir.dt.float32, kind="ExternalInput")
    idx = nc.dram_tensor("idx", [n_tiles, 128, NSLOT], mybir.dt.int32, kind="ExternalInput")
    dense = nc.dram_tensor("dense", [n_tiles * 128, D], mybir.dt.float32, kind="ExternalInput")
    out = nc.dram_tensor("out", [n_tiles * 128, NOUT], mybir.dt.float32, kind="ExternalOutput")
    with tile.TileContext(nc) as tc:
        with tc.tile_pool(name="rp", bufs=2) as rp, tc.tile_pool(name="ip", bufs=3) as ip:
            for i in range(n_tiles):
                it = ip.tile([128, NSLOT], mybir.dt.int32, tag="idx", name=f"it{i}")
                nc.sync.dma_start(out=it[:], in_=idx[i])
                R = rp.tile([128, N_BLK * D], mybir.dt.float32, tag="R", name=f"R{i}")
                nc.sync.dma_start(out=R[:, 0:D], in_=dense[i * 128:(i + 1) * 128, :])
                for s, (t, j, blk, first) in enumerate(SLOTS):
                    op = mybir.AluOpType.bypass if first else mybir.AluOpType.add
                    nc.gpsimd.indirect_dma_start(
                        out=R[:, blk * D:(blk + 1) * D], out_offset=None,
                        in_=w[:],
                        in_offset=bass.IndirectOffsetOnAxis(ap=it[:, s:s + 1], axis=0),
                        compute_op=op)
                for extra, main in ((27, 21), (28, 21), (29, 21), (30, 22)):
                    nc.vector.tensor_add(
                        out=R[:, main * D:(main + 1) * D],
                        in0=R[:, main * D:(main + 1) * D],
                        in1=R[:, extra * D:(extra + 1) * D])
                nc.sync.dma_start(out=out[i * 128:(i + 1) * 128, :], in_=R[:, 0:NOUT])
    return nc


# ---------------------------------------------------------------- runner
class BassRunner:
    def __init__(self, nc, n_cores):
        import jax
        from concourse import bass2jax
        bass2jax.install_neuronx_cc_hook()
        self.jax = jax
        self.nc = nc
        self.n_cores = n_cores
        partition_name = nc.partition_id_tensor.name if nc.partition_id_tensor else None
        in_names, out_names, out_avals, zero_outs = [], [], [], []
        for alloc in nc.m.functions[0].allocations:
            if not isinstance(alloc, mybir.MemoryLocationSet):
                continue
            name = alloc.memorylocations[0].name
            if alloc.kind == "ExternalInput":
                if name != partition_name:
                    in_names.append(name)
            elif alloc.kind == "ExternalOutput":
                out_names.append(name)
                shape = tuple(alloc.tensor_shape)
                dtype = mybir.dt.np(alloc.dtype)
                out_avals.append(jax.core.ShapedArray(shape, dtype))
                zero_outs.append(np.zeros(shape, dtype))
        self.in_names, self.out_names = in_names, out_names
        self.zero_outs = zero_outs
        all_in_names = list(in_names) + list(out_names)
        if partition_name is not None:
            all_in_names.append(partition_name)

        def _body(*args):
            operands = list(args)
            if partition_name is not None:
                operands.append(bass2jax.partition_id_tensor())
            return tuple(bass2jax._bass_exec_p.bind(
                *operands,
                out_avals=tuple(out_avals),
                in_names=tuple(all_in_names),
                out_names=tuple(out_names),
                lowering_input_output_aliases=(),
                sim_require_finite=True,
                sim_require_nnan=True,
                nc=nc))

        if n_cores == 1:
            self.jf = jax.jit(_body, keep_unused=True)
            self.mesh = None
        else:
            from jax.sharding import Mesh, PartitionSpec
            from jax.experimental.shard_map import shard_map
            devices = jax.devices()[:n_cores]
            assert len(devices) == n_cores
            self.mesh = Mesh(np.asarray(devices), ("core",))
            n = len(in_names) + len(out_names)
            self.jf = jax.jit(shard_map(
                _body, mesh=self.mesh,
                in_specs=(PartitionSpec("core"),) * n,
                out_specs=(PartitionSpec("core"),) * len(out_names),
                check_rep=False))

    def put_shards(self, per_core_arrays):
        """per_core_arrays: list (one per input name) of lists (one per core)
        of np arrays. Builds global sharded jax arrays without a host concat."""
        jax = self.jax
        if self.n_cores == 1:
            return [jax.device_put(a[0]) for a in per_core_arrays]
        from jax.sharding import NamedSharding, PartitionSpec
        sh = NamedSharding(self.mesh, PartitionSpec("core"))
        devices = list(self.mesh.devices)
        out = []
        for shards in per_core_arrays:
            s0 = np.asarray(shards[0])
            gshape = (s0.shape[0] * self.n_cores,) + s0.shape[1:]
            parts = [jax.device_put(np.asarray(shards[c]), devices[c])
                     for c in range(self.n_cores)]
            out.append(jax.make_array_from_single_device_arrays(gshape, sh, parts))
        return out

    def run(self, dev_args):
        outs = self.jf(*dev_args)
        self.jax.block_until_ready(outs)
        return outs

    def results(self, outs):
        res = [dict() for _ in range(self.n_cores)]
        for i, name in enumerate(self.out_names):
            full = np.asarray(outs[i])
            if self.n_cores == 1:
                res[0][name] = full
            else:
                for c, part in enumerate(np.split(full, self.n_cores, axis=0)):
                    res[c][name] = part
        return res


# ---------------------------------------------------------------- host packing
def pack_indices(index, B, n_cores):
    """-> [n_cores][n_tiles, 128, NSLOT] int32 globalized slot-ordered indices."""
    n_tiles = B // (128 * n_cores)
    gi = []
    for t in range(NUM_TABLES):
        mh = MULTI_HOT[t]
        a = np.asarray(index[t]).reshape(B, mh).astype(np.int64) + ROW_BASE[t]
        gi.append(a.astype(np.int32))
    per_core = []
    rows_per_core = B // n_cores
    for c in range(n_cores):
        arr = np.empty((n_tiles, 128, NSLOT), np.int32)
        r0 = c * rows_per_core
        for s, (t, j, blk, first) in enumerate(SLOTS):
            arr[:, :, s] = gi[t][r0:r0 + rows_per_core, j].reshape(n_tiles, 128)
        per_core.append(arr)
    return per_core


_CACHE = {}


def kernel(dense, weights, index, offsets):
    dense = np.ascontiguousarray(np.asarray(dense), dtype=np.float32)
    B = dense.shape[0]
    assert B % (128 * N_CORES) == 0, B
    n_tiles = B // (128 * N_CORES)

    key = ("runner", n_tiles)
    if key not in _CACHE:
        _CACHE[key] = BassRunner(build_program(n_tiles), N_CORES)
    runner = _CACHE[key]

    # merged weight table (cache by object identity of the weights tuple)
    wk = ("w", tuple(id(w) for w in weights))
    if wk not in _CACHE:
        w_all = np.empty((VTOT, D), np.float32)
        for t in range(NUM_TABLES):
            wt = np.asarray(weights[t])
            assert wt.shape == (POOL[t], D), (t, wt.shape)
            w_all[ROW_BASE[t]:ROW_BASE[t + 1]] = wt
        _CACHE[wk] = w_all
    w_all = _CACHE[wk]

    idx_cores = pack_indices(index, B, N_CORES)
    rows_per_core = B // N_CORES
    dense_cores = [dense[c * rows_per_core:(c + 1) * rows_per_core] for c in range(N_CORES)]

    per_core_arrays = [
        [w_all] * N_CORES,           # "w"   (replicated)
        idx_cores,                   # "idx"
        dense_cores,                 # "dense"
    ]
    # order must match runner.in_names; then zero outputs
    name_order = {n: i for i, n in enumerate(["w", "idx", "dense"])}
    arrays = [per_core_arrays[name_order[n]] for n in runner.in_names]
    arrays += [[z] * N_CORES for z in runner.zero_outs]
    dev = runner.put_shards(arrays)
    outs = runner.run(dev)
    res = runner.results(outs)
    return np.concatenate([res[c]["out"] for c in range(N_CORES)], axis=0)
